# revision 15
# baseline (speedup 1.0000x reference)
"""Trainium2 Bass kernel for GNN message passing (edge MLP + segment_sum + node MLP).

Strategy: edges are bucketed on host by receiver node-tile (128 nodes per tile,
6272 nodes per core, 8 cores).  One SPMD program runs on all 8 cores; per-core
column metadata is made uniform by padding each node-tile's edge bucket to the
max column count over cores.  On device, per 512-edge chunk:
  gather x[senders]/x[receivers] rows (indirect DMA) -> PE-transpose to
  feature-major -> 4-layer MLP on TensorE (fp32) -> transpose back ->
  LayerNorm via bn_stats -> one-hot matmul scatter-add into node aggregates.
Then a node MLP over concat(x, agg) per node shard.  No collectives needed.
"""

import math
import sys
from contextlib import ExitStack

import numpy as np

sys.path.insert(0, "/opt/trn_rl_repo")

import concourse.bass as bass  # noqa: E402
import concourse.bacc as bacc  # noqa: E402
import concourse.tile as tile  # noqa: E402
from concourse import mybir  # noqa: E402

P = 128
D = 128
LN_EPS = 1e-5
F32 = mybir.dt.float32
I32 = mybir.dt.int32
I16 = mybir.dt.int16
ALU = mybir.AluOpType
AF = mybir.ActivationFunctionType


# ----------------------------------------------------------------------------
# Host-side planning
# ----------------------------------------------------------------------------

HI_BASE = 32768


class Plan:
    def __init__(self, n_nodes, senders, receivers, n_cores=8):
        self.n_nodes = n_nodes
        self.n_cores = n_cores
        npc = int(math.ceil(n_nodes / n_cores / P)) * P  # nodes per core
        self.npc = npc
        self.tiles_per_core = npc // P
        nt = self.tiles_per_core

        core_of = receivers // npc
        local = receivers - core_of * npc
        tile_of = local // P
        self.rloc_of_edge = (local % P).astype(np.float32)
        hi_of = senders >= HI_BASE  # sender index kind (int16 range split)

        # per (core, tile, kind) counts
        cnt = np.zeros((n_cores, nt, 2), dtype=np.int64)
        np.add.at(cnt, (core_of, tile_of, hi_of.astype(np.int64)), 1)
        cols_k = (cnt + P - 1) // P  # ceil
        cols_k = cols_k.max(axis=0)  # [nt, 2] cross-core max
        empty = cols_k.sum(axis=1) == 0
        cols_k[empty, 0] = 1
        ncol = int(cols_k.sum())
        pad = (-ncol) % 4
        cols_k[nt - 1, 1] += pad  # pad as hi cols of last tile (sentinels)
        ncol += pad
        self.ncol = ncol
        self.has_hi = bool(cols_k[:, 1].sum() > 0) and n_nodes > HI_BASE

        tile_col0 = np.zeros(nt, dtype=np.int64)
        tile_col0[1:] = np.cumsum(cols_k.sum(axis=1))[:-1]
        self.tile_col0 = tile_col0
        self.cols_k = cols_k

        self.col_tile = np.zeros(ncol, dtype=np.int64)
        self.col_first = np.zeros(ncol, dtype=bool)
        self.col_last = np.zeros(ncol, dtype=bool)
        self.col_kind = np.zeros(ncol, dtype=np.int64)
        for t in range(nt):
            c0 = tile_col0[t]
            c1 = c0 + cols_k[t, 0] + cols_k[t, 1]
            self.col_tile[c0:c1] = t
            self.col_first[c0] = True
            self.col_last[c1 - 1] = True
            self.col_kind[c0 + cols_k[t, 0]:c1] = 1

        # per-chunk runs of same-kind columns for gather calls
        self.chunk_runs = []
        for ch in range(ncol // 4):
            runs = []
            g = 0
            while g < 4:
                k = self.col_kind[ch * 4 + g]
                g1 = g
                while g1 < 4 and self.col_kind[ch * 4 + g1] == k:
                    g1 += 1
                runs.append((g, g1, int(k)))
                g = g1
            self.chunk_runs.append(runs)

        # slot assignment per core
        self.core_edge_lists = []
        for c in range(n_cores):
            e_c = np.where(core_of == c)[0]
            key = tile_of[e_c] * 2 + hi_of[e_c]
            order = np.argsort(key, kind="stable")
            e_c = e_c[order]
            tt = tile_of[e_c]
            kk = hi_of[e_c].astype(np.int64)
            # j index within each (tile, kind) bucket
            key_s = key[order]
            bucket_start = np.searchsorted(key_s, np.arange(nt * 2))
            j_within = np.arange(len(e_c)) - bucket_start[key_s]
            col = tile_col0[tt] + kk * cols_k[tt, 0] + j_within // P
            p = j_within % P
            slot = col * P + p
            self.core_edge_lists.append((e_c, slot))


def _build_core_inputs(plan, c, x, edge_attr, senders, receivers, wts_np, bias_np,
                       consts_np, vrow_np, use_deg):
    ncol, npc = plan.ncol, plan.npc
    idx_s16 = np.zeros((16, ncol * 8), dtype=np.int16)
    idx_r16 = np.zeros((16, ncol * 8), dtype=np.int16)
    rloc = np.full((P, ncol), -1.0, dtype=np.float32)
    ea = np.zeros((P, ncol * P), dtype=np.float32)

    e_c, slot = plan.core_edge_lists[c]
    pp = slot % P
    cc = slot // P
    sval = senders[e_c] - HI_BASE * plan.col_kind[cc]
    rval = receivers[e_c] - c * npc
    rows16 = pp % 16
    cols16 = cc * 8 + pp // 16
    idx_s16[rows16, cols16] = sval.astype(np.int16)
    idx_r16[rows16, cols16] = rval.astype(np.int16)
    idx_s16 = np.tile(idx_s16, (8, 1))
    idx_r16 = np.tile(idx_r16, (8, 1))
    rloc[pp, cc] = plan.rloc_of_edge[e_c]
    ea[:, slot] = edge_attr[e_c].T

    n0 = c * npc
    n1 = min(plan.n_nodes, n0 + npc)
    xfm = np.zeros((P, npc), dtype=np.float32)
    xfm[:, : n1 - n0] = x[n0:n1].T
    xloc = np.zeros((npc, P), dtype=np.float32)
    xloc[: n1 - n0] = x[n0:n1]
    degfm = np.zeros((1, npc), dtype=np.float32)
    if use_deg:
        deg = np.bincount(receivers, minlength=plan.n_nodes).astype(np.float32)
        degfm[0, : n1 - n0] = deg[n0:n1]

    return {
        "xtab": np.ascontiguousarray(x),
        "xloc": xloc,
        "ea": ea,
        "idxs": idx_s16,
        "idxr": idx_r16,
        "rloc": rloc,
        "xfm": xfm,
        "degfm": degfm,
        "wts": wts_np,
        "bias": bias_np,
        "consts": consts_np,
        "vrow": vrow_np,
    }


# ----------------------------------------------------------------------------
# Device kernel
# ----------------------------------------------------------------------------

def _emit_kernel(nc, plan, use_deg):
    ncol = plan.ncol
    npc = plan.npc
    nt = plan.tiles_per_core
    n_nodes = plan.n_nodes

    xtab = nc.dram_tensor("xtab", [n_nodes, D], F32, kind="ExternalInput").ap()
    xloc_d = nc.dram_tensor("xloc", [npc, D], F32, kind="ExternalInput").ap()
    ea_d = nc.dram_tensor("ea", [P, ncol * P], F32, kind="ExternalInput").ap()
    idxs_d = nc.dram_tensor("idxs", [P, ncol * 8], I16,
                            kind="ExternalInput").ap()
    idxr_d = nc.dram_tensor("idxr", [P, ncol * 8], I16,
                            kind="ExternalInput").ap()
    rloc_d = nc.dram_tensor("rloc", [P, ncol], F32, kind="ExternalInput").ap()
    xfm_d = nc.dram_tensor("xfm", [P, npc], F32, kind="ExternalInput").ap()
    deg_d = nc.dram_tensor("degfm", [1, npc], F32, kind="ExternalInput").ap()
    wts_d = nc.dram_tensor("wts", [P, 11 * P], F32, kind="ExternalInput").ap()
    bias_d = nc.dram_tensor("bias", [P, 8], F32, kind="ExternalInput").ap()
    cst_d = nc.dram_tensor("consts", [P, 2 * P], F32, kind="ExternalInput").ap()
    vrow_d = nc.dram_tensor("vrow", [1, P], F32, kind="ExternalInput").ap()
    eout_d = nc.dram_tensor("eout", [ncol * P, D], F32, kind="ExternalOutput").ap()
    nout_d = nc.dram_tensor("nout", [npc, D], F32, kind="ExternalOutput").ap()

    with tile.TileContext(nc) as tc:
        with ExitStack() as ctx:
            _emit_body(ctx, tc, nc, plan, use_deg, xtab, xloc_d, ea_d, idxs_d,
                       idxr_d, rloc_d, xfm_d, deg_d, wts_d, bias_d, cst_d,
                       vrow_d, eout_d, nout_d)
    return nc


def _emit_body(ctx, tc, nc, plan, use_deg, xtab, xloc_d, ea_d, idxs_d, idxr_d,
               rloc_d, xfm_d, deg_d, wts_d, bias_d, cst_d, vrow_d, eout_d,
               nout_d):
    ncol, npc, nt = plan.ncol, plan.npc, plan.tiles_per_core

    from concourse import library_config
    nc.gpsimd.load_library(library_config.mlp)

    singles = ctx.enter_context(tc.tile_pool(name="singles", bufs=1))
    io_pool = ctx.enter_context(tc.tile_pool(name="io", bufs=3))
    idx_pool = ctx.enter_context(tc.tile_pool(name="idx", bufs=3))
    g_pool = ctx.enter_context(tc.tile_pool(name="gath", bufs=3))
    mid_pool = ctx.enter_context(tc.tile_pool(name="mid", bufs=2))
    st_pool = ctx.enter_context(tc.tile_pool(name="st", bufs=3))
    out_pool = ctx.enter_context(tc.tile_pool(name="out", bufs=3))
    psA = ctx.enter_context(tc.tile_pool(name="psA", bufs=3, space="PSUM"))
    psMM = ctx.enter_context(tc.tile_pool(name="psMM", bufs=3, space="PSUM"))
    psAgg = ctx.enter_context(tc.tile_pool(name="psAgg", bufs=2, space="PSUM"))

    # constants / persistent
    wts = singles.tile([P, 11 * P], F32)
    nc.sync.dma_start(out=wts[:], in_=wts_d[:, :])
    bias = singles.tile([P, 8], F32)
    nc.sync.dma_start(out=bias[:], in_=bias_d[:, :])
    csts = singles.tile([P, 2 * P], F32)
    nc.sync.dma_start(out=csts[:], in_=cst_d[:, :])
    iota_t = csts[:, 0:P]
    ident = csts[:, P:2 * P]
    eps_t = singles.tile([P, 1], F32)
    nc.vector.memset(eps_t[:], LN_EPS)
    agg_sb = singles.tile([P, nt * P], F32)
    xfm_sb = singles.tile([P, npc], F32)
    nc.sync.dma_start(out=xfm_sb[:], in_=xfm_d[:, :])
    if use_deg:
        deg_sb = singles.tile([1, npc], F32)
        nc.sync.dma_start(out=deg_sb[:], in_=deg_d[:, :])
        vrow_sb = singles.tile([1, P], F32)
        nc.sync.dma_start(out=vrow_sb[:], in_=vrow_d[:, :])

    def w(k):
        return wts[:, k * P:(k + 1) * P]

    W1S, W1R, W1E, W2, W3, W4, WN1X, WN1A, WN2, WN3, WN4 = range(11)

    def mlp_tail(h1p, ncols_e, b_off, out_sb_tag):
        """Layers 2-4 + LN for a chunk with ncols_e*128 elements in free dim.
        h1p: PSUM with L1 preactivation. Returns (enew_sb, em_ps)."""
        sz = ncols_e * P
        h1 = mid_pool.tile([P, 512], F32, tag="h1")
        nc.scalar.activation(out=h1[:, :sz], in_=h1p[:, :sz], func=AF.Relu,
                             bias=bias[:, b_off:b_off + 1], scale=1.0)
        h2p = psMM.tile([P, 512], F32, tag="mm")
        nc.tensor.matmul(out=h2p[:, :sz], lhsT=w(W2 if b_off == 0 else WN2),
                         rhs=h1[:, :sz], start=True, stop=True)
        h2 = mid_pool.tile([P, 512], F32, tag="h2")
        nc.vector.tensor_scalar(h2[:, :sz], h2p[:, :sz],
                                bias[:, b_off + 1:b_off + 2], 0.0,
                                ALU.add, ALU.max)
        h3p = psMM.tile([P, 512], F32, tag="mm")
        nc.tensor.matmul(out=h3p[:, :sz], lhsT=w(W3 if b_off == 0 else WN3),
                         rhs=h2[:, :sz], start=True, stop=True)
        h3 = mid_pool.tile([P, 512], F32, tag="h3")
        nc.scalar.activation(out=h3[:, :sz], in_=h3p[:, :sz], func=AF.Relu,
                             bias=bias[:, b_off + 2:b_off + 3], scale=1.0)
        h4p = psMM.tile([P, 512], F32, tag="mm")
        nc.tensor.matmul(out=h4p[:, :sz], lhsT=w(W4 if b_off == 0 else WN4),
                         rhs=h3[:, :sz], start=True, stop=True)
        h4 = mid_pool.tile([P, 512], F32, tag="h4")
        nc.vector.tensor_scalar(h4[:, :sz], h4p[:, :sz],
                                bias[:, b_off + 3:b_off + 4], None, ALU.add)
        # transpose back to edge/node-major + LN
        em_ps = psA.tile([P, 512], F32, tag="psA")
        for g in range(ncols_e):
            gs = slice(g * P, (g + 1) * P)
            nc.tensor.transpose(out=em_ps[:, gs], in_=h4[:, gs], identity=ident)
        st6 = st_pool.tile([P, 4, 6], F32, tag="st6")
        mv = st_pool.tile([P, 4, 2], F32, tag="mv")
        for g in range(ncols_e):
            gs = slice(g * P, (g + 1) * P)
            nc.vector.bn_stats(out=st6[:, g, :], in_=em_ps[:, gs])
            nc.vector.bn_aggr(out=mv[:, g, :], in_=st6[:, g, :])
        mean_ap = mv[:, :ncols_e, 0:1].rearrange("p a b -> p (a b)")
        var_ap = mv[:, :ncols_e, 1:2].rearrange("p a b -> p (a b)")
        sd = st_pool.tile([P, 4], F32, tag="sd")
        nc.scalar.activation(out=sd[:, :ncols_e], in_=var_ap, func=AF.Sqrt,
                             bias=eps_t[:, 0:1], scale=1.0)
        S = st_pool.tile([P, 4], F32, tag="S")
        nc.vector.reciprocal(out=S[:, :ncols_e], in_=sd[:, :ncols_e])
        nB = st_pool.tile([P, 4], F32, tag="nB")
        nc.vector.tensor_tensor(out=nB[:, :ncols_e], in0=mean_ap,
                                in1=S[:, :ncols_e], op=ALU.mult)
        nc.vector.tensor_scalar(nB[:, :ncols_e], nB[:, :ncols_e], -1.0, None,
                                ALU.mult)
        enew = out_pool.tile([P, 512], F32, tag=out_sb_tag)
        for g in range(ncols_e):
            gs = slice(g * P, (g + 1) * P)
            nc.scalar.activation(out=enew[:, gs], in_=em_ps[:, gs],
                                 func=AF.Identity, bias=nB[:, g:g + 1],
                                 scale=S[:, g:g + 1])
        return enew

    # ---------------- edge phase ----------------
    n_chunks = ncol // 4
    cur_agg = [None]
    xtab_lo = xtab
    xtab_hi = xtab[HI_BASE:, :] if plan.has_hi else xtab

    for ch in range(n_chunks):
        c4 = ch * 4
        ea_t = io_pool.tile([P, 512], F32, tag="ea")
        nc.sync.dma_start(out=ea_t[:], in_=ea_d[:, c4 * P:(c4 + 4) * P])
        idx_s = idx_pool.tile([P, 32], I16, tag="idxs")
        nc.sync.dma_start(out=idx_s[:], in_=idxs_d[:, c4 * 8:(c4 + 4) * 8])
        idx_r = idx_pool.tile([P, 32], I16, tag="idxr")
        nc.sync.dma_start(out=idx_r[:], in_=idxr_d[:, c4 * 8:(c4 + 4) * 8])
        rloc_t = idx_pool.tile([P, 4], F32, tag="rloc")
        nc.sync.dma_start(out=rloc_t[:], in_=rloc_d[:, c4:c4 + 4])

        xs_em = g_pool.tile([P, 4, D], F32, tag="xs")
        for (g0, g1, kind) in plan.chunk_runs[ch]:
            num = (g1 - g0) * P
            nc.gpsimd.dma_gather(
                out_ap=xs_em[:, g0:g1, :],
                in_ap=xtab_hi if kind else xtab_lo,
                idxs_ap=idx_s[:, g0 * 8:g1 * 8],
                num_idxs=num, num_idxs_reg=num, elem_size=D)
        xr_em = g_pool.tile([P, 4, D], F32, tag="xr")
        nc.gpsimd.dma_gather(
            out_ap=xr_em[:], in_ap=xloc_d, idxs_ap=idx_r[:],
            num_idxs=512, num_idxs_reg=512, elem_size=D)

        xs_ps = psA.tile([P, 512], F32, tag="psA")
        for g in range(4):
            nc.tensor.transpose(out=xs_ps[:, g * P:(g + 1) * P],
                                in_=xs_em[:, g, :], identity=ident)
        xs_fm = mid_pool.tile([P, 512], F32, tag="xsfm")
        nc.scalar.copy(out=xs_fm[:], in_=xs_ps[:])
        xr_ps = psA.tile([P, 512], F32, tag="psA")
        for g in range(4):
            nc.tensor.transpose(out=xr_ps[:, g * P:(g + 1) * P],
                                in_=xr_em[:, g, :], identity=ident)
        xr_fm = mid_pool.tile([P, 512], F32, tag="xrfm")
        nc.vector.tensor_copy(out=xr_fm[:], in_=xr_ps[:])

        h1p = psMM.tile([P, 512], F32, tag="mm")
        nc.tensor.matmul(out=h1p[:], lhsT=w(W1S), rhs=xs_fm[:], start=True,
                         stop=False)
        nc.tensor.matmul(out=h1p[:], lhsT=w(W1R), rhs=xr_fm[:], start=False,
                         stop=False)
        nc.tensor.matmul(out=h1p[:], lhsT=w(W1E), rhs=ea_t[:], start=False,
                         stop=True)

        enew = mlp_tail(h1p, 4, 0, "enew")

        oh = out_pool.tile([P, 512], F32, tag="oh")
        for g in range(4):
            gs = slice(g * P, (g + 1) * P)
            nc.vector.tensor_tensor(
                out=oh[:, gs],
                in0=rloc_t[:, g:g + 1].to_broadcast([P, P]),
                in1=iota_t, op=ALU.is_equal)
        for g in range(4):
            col = c4 + g
            gs = slice(g * P, (g + 1) * P)
            t = int(plan.col_tile[col])
            if plan.col_first[col]:
                aggps = psAgg.tile([P, P], F32, tag="agg")
                cur_agg[0] = aggps
            nc.tensor.matmul(out=cur_agg[0][:], lhsT=oh[:, gs],
                             rhs=enew[:, gs], start=bool(plan.col_first[col]),
                             stop=bool(plan.col_last[col]))
            if plan.col_last[col]:
                nc.vector.tensor_copy(out=agg_sb[:, t * P:(t + 1) * P],
                                      in_=cur_agg[0][:])

        eo_ap = eout_d[c4 * P:(c4 + 4) * P, :].rearrange(
            "(g p) f -> p g f", p=P)
        nc.sync.dma_start(out=eo_ap,
                          in_=enew[:].rearrange("p (g f) -> p g f", g=4))

    # ---------------- node phase ----------------
    aggfm = singles.tile([P, npc], F32)
    for t in range(nt):
        tp = psA.tile([P, 512], F32, tag="psA")
        nc.tensor.transpose(out=tp[:, 0:P], in_=agg_sb[:, t * P:(t + 1) * P],
                            identity=ident)
        if t % 2 == 0:
            nc.scalar.copy(out=aggfm[:, t * P:(t + 1) * P], in_=tp[:, 0:P])
        else:
            nc.vector.tensor_copy(out=aggfm[:, t * P:(t + 1) * P],
                                  in_=tp[:, 0:P])

    pos = 0
    while pos < npc:
        sz = min(512, npc - pos)
        ncols_e = sz // P
        sl = slice(pos, pos + sz)
        h1p = psMM.tile([P, 512], F32, tag="mm")
        nc.tensor.matmul(out=h1p[:, :sz], lhsT=w(WN1X), rhs=xfm_sb[:, sl],
                         start=True, stop=False)
        nc.tensor.matmul(out=h1p[:, :sz], lhsT=w(WN1A), rhs=aggfm[:, sl],
                         start=False, stop=not use_deg)
        if use_deg:
            nc.tensor.matmul(out=h1p[:, :sz], lhsT=vrow_sb[:, :],
                             rhs=deg_sb[:, sl], start=False, stop=True)
        xnew = mlp_tail(h1p, ncols_e, 4, "xnew")
        no_ap = nout_d[pos:pos + sz, :].rearrange("(g p) f -> p g f", p=P)
        nc.sync.dma_start(
            out=no_ap,
            in_=xnew[:, :sz].rearrange("p (g f) -> p g f", g=ncols_e))
        pos += sz


# ----------------------------------------------------------------------------
# Public entry point
# ----------------------------------------------------------------------------

def _prep(x, edge_attr, edge_index, params):
    x = np.asarray(x, dtype=np.float32)
    edge_attr = np.asarray(edge_attr, dtype=np.float32)
    senders = np.asarray(edge_index[0], dtype=np.int64)
    receivers = np.asarray(edge_index[1], dtype=np.int64)
    n_nodes = x.shape[0]
    plan = Plan(n_nodes, senders, receivers)

    pe, pn = params["edge"], params["node"]
    ge = np.asarray(pe["g"], np.float32)
    be = np.asarray(pe["beta"], np.float32)
    gn = np.asarray(pn["g"], np.float32)
    bn_ = np.asarray(pn["beta"], np.float32)
    ident_affine = (np.allclose(ge, 1) and np.allclose(be, 0)
                    and np.allclose(gn, 1) and np.allclose(bn_, 0))

    eW = [np.asarray(m, np.float32) for m in pe["W"]]
    nW = [np.asarray(m, np.float32) for m in pn["W"]]
    eb = [np.asarray(m, np.float32) for m in pe["b"]]
    nb = [np.asarray(m, np.float32) for m in pn["b"]]

    wn1 = nW[0]  # [256, 128]
    wn1x, wn1a = wn1[:D], wn1[D:]
    if not ident_affine:
        wn1a = (ge[:, None] * wn1a).astype(np.float32)
    wts_np = np.concatenate(
        [eW[0][:D], eW[0][D:2 * D], eW[0][2 * D:], eW[1], eW[2], eW[3],
         wn1x, wn1a, nW[1], nW[2], nW[3]], axis=1).astype(np.float32)
    assert wts_np.shape == (P, 11 * P)
    bias_np = np.stack(eb + nb, axis=1).astype(np.float32)
    iota = np.tile(np.arange(P, dtype=np.float32), (P, 1))
    ident = np.eye(P, dtype=np.float32)
    consts_np = np.concatenate([iota, ident], axis=1)
    vrow_np = (be @ wn1a).reshape(1, P).astype(np.float32) \
        if not ident_affine else np.zeros((1, P), np.float32)

    in_maps = [
        _build_core_inputs(plan, c, x, edge_attr, senders, receivers, wts_np,
                           bias_np, consts_np, vrow_np, not ident_affine)
        for c in range(plan.n_cores)
    ]
    affine = None if ident_affine else (ge, be, gn, bn_)
    return plan, in_maps, affine


def _assemble(plan, results, senders_shape_E, affine):
    eouts = [r["eout"] for r in results]
    nouts = [r["nout"] for r in results]
    E = senders_shape_E
    edge_new = np.empty((E, D), dtype=np.float32)
    for c in range(plan.n_cores):
        e_c, slot = plan.core_edge_lists[c]
        edge_new[e_c] = eouts[c][slot]
    x_new = np.empty((plan.n_nodes, D), dtype=np.float32)
    for c in range(plan.n_cores):
        n0 = c * plan.npc
        n1 = min(plan.n_nodes, n0 + plan.npc)
        x_new[n0:n1] = nouts[c][: n1 - n0]
    if affine is not None:
        ge, be, gn, bn_ = affine
        edge_new = edge_new * ge + be
        x_new = x_new * gn + bn_
    return x_new, edge_new


_CACHE = {}


def kernel(x, edge_attr, edge_index, params):
    from concourse.bass_utils import run_bass_kernel_spmd

    plan, in_maps, affine = _prep(x, edge_attr, edge_index, params)
    nc = bacc.Bacc("TRN2", target_bir_lowering=False, debug=False,
                   num_devices=plan.n_cores)
    _emit_kernel(nc, plan, affine is not None)
    nc.compile()
    core_ids = list(range(plan.n_cores))
    res = run_bass_kernel_spmd(nc, in_maps, core_ids)
    x_new, edge_new = _assemble(plan, res.results,
                                np.asarray(edge_index).shape[1], affine)
    return x_new, edge_new
